# revision 7
# baseline (speedup 1.0000x reference)
"""Trainium2 Bass kernel for nn_BeamDecoder: 10-beam search, 30 steps,
LSTM decoder + attention + vocab projection (V=50257), 8-way vocab-sharded.

Self-contained: hardcodes all shapes/sharding. kernel(**inputs) takes the
full unsharded inputs (as produced by setup_inputs) and returns
(seqs int32 [10,31], scores f32 [10], norm_scores f32 [10]).
"""
import os
import numpy as np

import concourse.bass as bass
import concourse.bacc as bacc
import concourse.mybir as mybir
import concourse.tile as tile
from concourse import bass_utils

F32 = mybir.dt.float32
I32 = mybir.dt.int32
U32 = mybir.dt.uint32
I16 = mybir.dt.int16
AF = mybir.ActivationFunctionType
OP = mybir.AluOpType

V, E, H1, H2, Q = 50257, 128, 128, 64, 64
K, T, MAXL = 10, 2048, 30
SOS, EOS, ALPHA = 1, 2, 1.2
NEG = -1e30
NC_ = 8          # cores
VS = 6400        # vocab shard per core (padded: 8*6400 = 51200 >= V)
NTILE = VS // 128  # 50 vocab tiles per core
NSTEPS = int(os.environ.get("BEAM_STEPS", str(MAXL)))

_cache = {}


def _build(nsteps: int):
    nc = bacc.Bacc("TRN2", target_bir_lowering=False, debug=False, num_devices=NC_)

    def inp(name, shape, dtype=F32):
        return nc.declare_dram_parameter(name, list(shape), dtype, isOutput=False)

    wct = inp("wct", [128, VS])
    bct = inp("bct", [128, NTILE])
    offp = inp("offp", [128, 1])
    keyt = inp("keyt", [64, T])
    valt = inp("valt", [128, 16 * 64])
    w1a = inp("w1a", [128, 512])
    w1b = inp("w1b", [65, 512])
    w1c = inp("w1c", [128, 512])
    w2a = inp("w2a", [128, 256])
    w2b = inp("w2b", [65, 256])
    wqt = inp("wqt", [65, 64])
    emb = inp("emb", [V, E])
    i10 = inp("i10", [10, 10])
    iota10 = inp("iota10", [10, 10])
    ones128 = inp("ones128", [128, 1])
    onesrow = inp("onesrow", [1, 128])
    ones1 = inp("ones1", [1, 1])
    sosidx = inp("sosidx", [10, 1], U32)
    negstep0 = inp("negstep0", [10, 1])

    seqs_o = nc.declare_dram_parameter("seqs_o", [10, MAXL + 1], F32, isOutput=True)
    scores_o = nc.declare_dram_parameter("scores_o", [1, 10], F32, isOutput=True)

    with tile.TileContext(nc) as tc:
        with (
            tc.tile_pool(name="cst", bufs=1) as cst,
            tc.tile_pool(name="st", bufs=2) as st,
            tc.tile_pool(name="wk", bufs=2) as wk,
            tc.tile_pool(name="wkb", bufs=2) as wkb,
            tc.tile_pool(name="psb", bufs=2, space="PSUM") as psb,
            tc.tile_pool(name="psg", bufs=1, space="PSUM") as psg,
            tc.tile_pool(name="pss", bufs=3, space="PSUM") as pss,
            tc.tile_pool(name="pst", bufs=2, space="PSUM") as pst,
            tc.tile_pool(name="dr", bufs=3, space="DRAM") as dr,
        ):
            def load(t_in, shape, dtype=F32):
                s = cst.tile(list(shape), dtype, tag="cst_" + t_in.name)
                nc.sync.dma_start(out=s[:], in_=t_in[:])
                return s

            wct_s = load(wct, [128, VS])
            bct_s = load(bct, [128, NTILE])
            offp_s = load(offp, [128, 1])
            keyt_s = load(keyt, [64, T])
            valt_s = load(valt, [128, 16 * 64])
            w1a_s = load(w1a, [128, 512])
            w1b_s = load(w1b, [65, 512])
            w1c_s = load(w1c, [128, 512])
            w2a_s = load(w2a, [128, 256])
            w2b_s = load(w2b, [65, 256])
            wqt_s = load(wqt, [65, 64])
            i10_s = load(i10, [10, 10])
            iota10_s = load(iota10, [10, 10])
            ones128_s = load(ones128, [128, 1])
            onesrow_s = load(onesrow, [1, 128])
            ones1_s = load(ones1, [1, 1])
            sosidx_s = load(sosidx, [10, 1], U32)
            negstep0_s = load(negstep0, [10, 1])

            # derived const: 1.0 at (core 0, partition 2) == global vocab id EOS
            p2mask = cst.tile([128, 1], F32)
            nc.vector.tensor_scalar(out=p2mask[:], in0=offp_s[:], scalar1=float(EOS),
                                    scalar2=None, op0=OP.is_equal)

            z10x128 = cst.tile([10, 128], F32)
            nc.vector.memset(z10x128[:], 0.0)
            z10x64 = cst.tile([10, 64], F32)
            nc.vector.memset(z10x64[:], 0.0)
            zT128 = cst.tile([128, 10], F32)
            nc.vector.memset(zT128[:], 0.0)
            zcol = cst.tile([10, 1], F32)
            nc.vector.memset(zcol[:], 0.0)
            zrow = cst.tile([1, 10], F32)
            nc.vector.memset(zrow[:], 0.0)
            ct0aug = cst.tile([65, 10], F32)
            nc.vector.memset(ct0aug[:], 0.0)
            nc.vector.memset(ct0aug[64:65, :], 1.0)
            h20aug = cst.tile([65, 10], F32)
            nc.vector.memset(h20aug[:], 0.0)
            nc.vector.memset(h20aug[64:65, :], 1.0)

            h1 = st.tile([10, 128], F32, tag="h1")
            nc.vector.memset(h1[:], 0.0)
            c1 = st.tile([10, 128], F32, tag="c1")
            nc.vector.memset(c1[:], 0.0)
            hc = st.tile([10, 128], F32, tag="hc")
            nc.vector.memset(hc[:], 0.0)
            c2 = st.tile([10, 64], F32, tag="c2")
            nc.vector.memset(c2[:], 0.0)
            seqs = st.tile([10, MAXL + 1], F32, tag="seqs")
            nc.vector.memset(seqs[:], float(SOS))
            fin_col = zcol
            fin_row = zrow
            scores_col = zcol
            scores_row = zrow

            V1 = None
            C16 = None

            def flat_topk(Vf, C16f, n, pfx):
                wv = wk.tile([1, 16], F32, tag=pfx + "wv")
                i1 = wk.tile([1, 8], U32, tag=pfx + "i1")
                i2 = wk.tile([1, 8], U32, tag=pfx + "i2")
                nc.vector.max(wv[:, 0:8], Vf[:])
                nc.vector.max_index(i1[:], wv[:, 0:8], Vf[:])
                vb = wk.tile([1, n], F32, tag=pfx + "vb")
                nc.vector.match_replace(vb[:], wv[:, 0:8], Vf[:], NEG)
                nc.vector.max(wv[:, 8:16], vb[:])
                nc.vector.max_index(i2[:], wv[:, 8:16], vb[:])
                posf = wk.tile([1, 16], F32, tag=pfx + "posf")
                nc.vector.tensor_copy(out=posf[:, 0:8], in_=i1[:])
                nc.vector.tensor_copy(out=posf[:, 8:16], in_=i2[:])
                posT = pst.tile([16, 1], F32, tag="ps_ty")
                nc.tensor.transpose(posT[:], posf[:], ones1_s[:])
                posi = wk.tile([16, 1], I16, tag=pfx + "posi")
                nc.vector.tensor_copy(out=posi[:], in_=posT[:])
                c16o = wk.tile([16, 16], F32, tag=pfx + "c16o")
                nc.gpsimd.ap_gather(c16o[:], C16f[:], posi[:],
                                    channels=16, num_elems=n, d=1, num_idxs=16)
                return wv, c16o

            def merge(i):
                nonlocal fin_col, fin_row, scores_col, scores_row, seqs
                wvals, codes16 = flat_topk(V1, C16, 128, "g")
                code10 = codes16[0:1, 0:10]
                bf_raw = wk.tile([1, 10], F32, tag="bfraw")
                nc.vector.tensor_scalar(out=bf_raw[:], in0=code10, scalar1=1.0 / V,
                                        scalar2=0.5 / V - 0.5, op0=OP.mult, op1=OP.add)
                bi = wk.tile([1, 10], I32, tag="bi")
                nc.vector.tensor_copy(out=bi[:], in_=bf_raw[:])
                b_f = wk.tile([1, 10], F32, tag="b_f")
                nc.vector.tensor_copy(out=b_f[:], in_=bi[:])
                v_row = wk.tile([1, 10], F32, tag="v_row")
                nc.vector.scalar_tensor_tensor(out=v_row[:], in0=b_f[:], scalar=float(-V),
                                               in1=code10, op0=OP.mult, op1=OP.add)
                sc = st.tile([10, 1], F32, tag="scol")
                scT = pst.tile([10, 1], F32, tag="ps_ty")
                nc.tensor.transpose(scT[:], wvals[:, 0:10], ones1_s[:])
                nc.vector.tensor_copy(out=sc[:], in_=scT[:])
                srow = st.tile([1, 10], F32, tag="srow")
                nc.vector.tensor_copy(out=srow[:], in_=wvals[:, 0:10])
                pbT = pst.tile([10, 1], F32, tag="ps_ty")
                nc.tensor.transpose(pbT[:], b_f[:], ones1_s[:])
                pb_col = wk.tile([10, 1], F32, tag="pbcol")
                nc.vector.tensor_copy(out=pb_col[:], in_=pbT[:])
                PT = wk.tile([10, 10], F32, tag="PT")
                nc.vector.tensor_tensor(out=PT[:], in0=pb_col[:].to_broadcast([10, 10]),
                                        in1=iota10_s[:], op=OP.is_equal)
                Pp = pss.tile([10, 10], F32, tag="ps_sm")
                nc.tensor.transpose(Pp[:], PT[:], i10_s[:])
                P_s = wk.tile([10, 10], F32, tag="P_s")
                nc.vector.tensor_copy(out=P_s[:], in_=Pp[:])
                eos_eq = wk.tile([1, 10], F32, tag="eoseq")
                nc.vector.tensor_scalar(out=eos_eq[:], in0=v_row[:], scalar1=float(EOS),
                                        scalar2=None, op0=OP.is_equal)
                fing = pst.tile([1, 10], F32, tag="ps_ty")
                nc.tensor.matmul(fing[:], fin_col[:], P_s[:], start=True, stop=True)
                fr = st.tile([1, 10], F32, tag="frow")
                nc.vector.tensor_tensor(out=fr[:], in0=fing[:], in1=eos_eq[:], op=OP.max)
                fT = pst.tile([10, 1], F32, tag="ps_ty")
                nc.tensor.transpose(fT[:], fr[:], ones1_s[:])
                fc = st.tile([10, 1], F32, tag="fcol")
                nc.vector.tensor_copy(out=fc[:], in_=fT[:])
                nxT = pst.tile([10, 1], F32, tag="ps_ty")
                nc.tensor.transpose(nxT[:], v_row[:], ones1_s[:])
                nxt_f = wk.tile([10, 1], F32, tag="nxtf")
                nc.vector.tensor_copy(out=nxt_f[:], in_=nxT[:])
                nxt_u = wk.tile([10, 1], U32, tag="nxtu")
                nc.vector.tensor_copy(out=nxt_u[:], in_=nxT[:])
                sq = pss.tile([10, MAXL + 1], F32, tag="ps_sm")
                nc.tensor.matmul(sq[:], P_s[:], seqs[:], start=True, stop=True)
                seqs_n = st.tile([10, MAXL + 1], F32, tag="seqs")
                nc.vector.tensor_copy(out=seqs_n[:], in_=sq[:])
                nc.vector.tensor_copy(out=seqs_n[:, i:i + 1], in_=nxt_f[:])
                fin_col, fin_row = fc, fr
                scores_col, scores_row = sc, srow
                seqs = seqs_n
                return P_s, nxt_u, wvals

            for i in range(nsteps):
                if i == 0:
                    P_s = None
                    embidx = sosidx_s
                else:
                    P_s, embidx, _ = merge(i)

                embrows = wk.tile([10, 128], F32, tag="embrows")
                nc.gpsimd.indirect_dma_start(
                    out=embrows[:], out_offset=None, in_=emb[:],
                    in_offset=bass.IndirectOffsetOnAxis(ap=embidx[:, 0:1], axis=0))
                embTp = pss.tile([128, 10], F32, tag="ps_sm")
                nc.tensor.transpose(embTp[:], embrows[:], i10_s[:])
                embT_s = wk.tile([128, 10], F32, tag="embT")
                nc.vector.tensor_copy(out=embT_s[:], in_=embTp[:])

                if i == 0:
                    h1gT_s, c1g_src = zT128, z10x128
                    c2g_src = z10x64
                    ctxaug, h2aug = ct0aug, h20aug
                else:
                    h1gTp = pss.tile([128, 10], F32, tag="ps_sm")
                    nc.tensor.matmul(h1gTp[:], h1[:], P_s[:], start=True, stop=True)
                    h1gT_s = wk.tile([128, 10], F32, tag="h1gT")
                    nc.vector.tensor_copy(out=h1gT_s[:], in_=h1gTp[:])
                    c1gp = pss.tile([10, 128], F32, tag="ps_sm")
                    nc.tensor.matmul(c1gp[:], P_s[:], c1[:], start=True, stop=True)
                    c1g_src = c1gp
                    c2gp = pss.tile([10, 64], F32, tag="ps_sm")
                    nc.tensor.matmul(c2gp[:], P_s[:], c2[:], start=True, stop=True)
                    c2g_src = c2gp
                    ctxgTp = pss.tile([64, 10], F32, tag="ps_sm")
                    nc.tensor.matmul(ctxgTp[:], hc[:, 64:128], P_s[:], start=True, stop=True)
                    ctxaug = wk.tile([65, 10], F32, tag="ctxaug")
                    nc.vector.memset(ctxaug[64:65, :], 1.0)
                    nc.vector.tensor_copy(out=ctxaug[0:64, :], in_=ctxgTp[:])
                    h2gTp = pss.tile([64, 10], F32, tag="ps_sm")
                    nc.tensor.matmul(h2gTp[:], hc[:, 0:64], P_s[:], start=True, stop=True)
                    h2aug = wk.tile([65, 10], F32, tag="h2aug")
                    nc.vector.memset(h2aug[64:65, :], 1.0)
                    nc.vector.tensor_copy(out=h2aug[0:64, :], in_=h2gTp[:])

                g1 = psg.tile([10, 512], F32, tag="gates")
                nc.tensor.matmul(g1[:], embT_s[:], w1a_s[:], start=True, stop=False)
                nc.tensor.matmul(g1[:], ctxaug[:], w1b_s[:], start=False, stop=False)
                nc.tensor.matmul(g1[:], h1gT_s[:], w1c_s[:], start=False, stop=True)
                si = wk.tile([10, 128], F32, tag="si")
                nc.scalar.activation(si[:], g1[:, 0:128], AF.Sigmoid)
                sf = wk.tile([10, 128], F32, tag="sf")
                nc.scalar.activation(sf[:], g1[:, 128:256], AF.Sigmoid)
                tg = wk.tile([10, 128], F32, tag="tg")
                nc.scalar.activation(tg[:], g1[:, 256:384], AF.Tanh)
                so = wk.tile([10, 128], F32, tag="so")
                nc.scalar.activation(so[:], g1[:, 384:512], AF.Sigmoid)
                m1t = wk.tile([10, 128], F32, tag="m1t")
                nc.vector.tensor_tensor(out=m1t[:], in0=sf[:], in1=c1g_src[:], op=OP.mult)
                m2t = wk.tile([10, 128], F32, tag="m2t")
                nc.vector.tensor_tensor(out=m2t[:], in0=si[:], in1=tg[:], op=OP.mult)
                c1n = st.tile([10, 128], F32, tag="c1")
                nc.vector.tensor_tensor(out=c1n[:], in0=m1t[:], in1=m2t[:], op=OP.add)
                tc1 = wk.tile([10, 128], F32, tag="tc1")
                nc.scalar.activation(tc1[:], c1n[:], AF.Tanh)
                h1n = st.tile([10, 128], F32, tag="h1")
                nc.vector.tensor_tensor(out=h1n[:], in0=so[:], in1=tc1[:], op=OP.mult)

                h1nTp = pss.tile([128, 10], F32, tag="ps_sm")
                nc.tensor.transpose(h1nTp[:], h1n[:], i10_s[:])
                h1nT_s = wk.tile([128, 10], F32, tag="h1nT")
                nc.vector.tensor_copy(out=h1nT_s[:], in_=h1nTp[:])
                g2 = psg.tile([10, 256], F32, tag="gates")
                nc.tensor.matmul(g2[:], h1nT_s[:], w2a_s[:], start=True, stop=False)
                nc.tensor.matmul(g2[:], h2aug[:], w2b_s[:], start=False, stop=True)
                si2 = wk.tile([10, 64], F32, tag="si2")
                nc.scalar.activation(si2[:], g2[:, 0:64], AF.Sigmoid)
                sf2 = wk.tile([10, 64], F32, tag="sf2")
                nc.scalar.activation(sf2[:], g2[:, 64:128], AF.Sigmoid)
                tg2 = wk.tile([10, 64], F32, tag="tg2")
                nc.scalar.activation(tg2[:], g2[:, 128:192], AF.Tanh)
                so2 = wk.tile([10, 64], F32, tag="so2")
                nc.scalar.activation(so2[:], g2[:, 192:256], AF.Sigmoid)
                m1u = wk.tile([10, 64], F32, tag="m1u")
                nc.vector.tensor_tensor(out=m1u[:], in0=sf2[:], in1=c2g_src[:], op=OP.mult)
                m2u = wk.tile([10, 64], F32, tag="m2u")
                nc.vector.tensor_tensor(out=m2u[:], in0=si2[:], in1=tg2[:], op=OP.mult)
                c2n = st.tile([10, 64], F32, tag="c2")
                nc.vector.tensor_tensor(out=c2n[:], in0=m1u[:], in1=m2u[:], op=OP.add)
                tc2 = wk.tile([10, 64], F32, tag="tc2")
                nc.scalar.activation(tc2[:], c2n[:], AF.Tanh)
                hc_n = st.tile([10, 128], F32, tag="hc")
                nc.vector.tensor_tensor(out=hc_n[:, 0:64], in0=so2[:], in1=tc2[:], op=OP.mult)

                h2Tp = pss.tile([64, 10], F32, tag="ps_sm")
                nc.tensor.transpose(h2Tp[:], hc_n[:, 0:64], i10_s[:])
                h2qaug = wk.tile([65, 10], F32, tag="h2qaug")
                nc.vector.memset(h2qaug[64:65, :], 1.0)
                nc.vector.tensor_copy(out=h2qaug[0:64, :], in_=h2Tp[:])
                qTp = pss.tile([64, 10], F32, tag="ps_sm")
                nc.tensor.matmul(qTp[:], wqt_s[:], h2qaug[:], start=True, stop=True)
                qT_s = wk.tile([64, 10], F32, tag="qT")
                nc.vector.tensor_copy(out=qT_s[:], in_=qTp[:])

                eng = psb.tile([128, 160], F32, tag="ps_big")
                for t in range(16):
                    nc.tensor.matmul(eng[:, 10 * t:10 * t + 10],
                                     keyt_s[:, 128 * t:128 * t + 128], qT_s[:],
                                     start=True, stop=True)
                e = wkb.tile([128, 160], F32, tag="e")
                nc.scalar.activation(e[:], eng[:], AF.Exp)
                esum_p = wk.tile([128, 10], F32, tag="esump")
                nc.vector.tensor_reduce(esum_p[:],
                                        e[:].rearrange("p (t b) -> p b t", b=10),
                                        mybir.AxisListType.X, OP.add)
                esumb = pst.tile([10, 1], F32, tag="ps_ty")
                nc.tensor.matmul(esumb[:], esum_p[:], ones128_s[:], start=True, stop=True)
                rc = wk.tile([10, 1], F32, tag="rc")
                nc.vector.reciprocal(rc[:], esumb[:])
                ctxu = pss.tile([10, 64], F32, tag="ps_sm")
                for t in range(16):
                    nc.tensor.matmul(ctxu[:], e[:, 10 * t:10 * t + 10],
                                     valt_s[:, 64 * t:64 * t + 64],
                                     start=(t == 0), stop=(t == 15))
                nc.vector.tensor_scalar(out=hc_n[:, 64:128], in0=ctxu[:], scalar1=rc[:],
                                        scalar2=None, op0=OP.mult)
                xcTp = pss.tile([128, 10], F32, tag="ps_sm")
                nc.tensor.transpose(xcTp[:], hc_n[:], i10_s[:])
                xcT_s = wk.tile([128, 10], F32, tag="xcT")
                nc.vector.tensor_copy(out=xcT_s[:], in_=xcTp[:])

                lg = psb.tile([128, 500], F32, tag="ps_big")
                for t in range(NTILE):
                    nc.tensor.matmul(lg[:, 10 * t:10 * t + 10],
                                     wct_s[:, 128 * t:128 * t + 128], xcT_s[:],
                                     start=True, stop=True)
                cb = wkb.tile([128, 500], F32, tag="cb")
                nc.vector.tensor_tensor(
                    out=cb[:].rearrange("p (t b) -> p t b", t=NTILE),
                    in0=lg[:].rearrange("p (t b) -> p t b", t=NTILE),
                    in1=bct_s[:].rearrange("p (t o) -> p t o", o=1).to_broadcast([128, NTILE, 10]),
                    op=OP.add)
                e2 = wkb.tile([128, 500], F32, tag="e2")
                nc.scalar.activation(e2[:], cb[:], AF.Exp)
                z_p = wk.tile([128, 10], F32, tag="zp")
                nc.vector.tensor_reduce(z_p[:],
                                        e2[:].rearrange("p (t b) -> p b t", b=10),
                                        mybir.AxisListType.X, OP.add)
                zb = pst.tile([10, 1], F32, tag="ps_ty")
                nc.tensor.matmul(zb[:], z_p[:], ones128_s[:], start=True, stop=True)
                zloc = wk.tile([10, 1], F32, tag="zloc")
                nc.vector.tensor_copy(out=zloc[:], in_=zb[:])
                zin = dr.tile([10, 1], F32, tag="zin")
                nc.sync.dma_start(out=zin[:], in_=zloc[:])
                zout = dr.tile([10, 1], F32, tag="zout")
                nc.gpsimd.collective_compute(
                    "AllReduce", OP.add,
                    replica_groups=[list(range(NC_))],
                    ins=[zin.opt()], outs=[zout.opt()])
                zg = wk.tile([10, 1], F32, tag="zg")
                nc.sync.dma_start(out=zg[:], in_=zout[:])
                lse = wk.tile([10, 1], F32, tag="lse")
                nc.scalar.activation(lse[:], zg[:], AF.Ln)
                shift = wk.tile([10, 1], F32, tag="shift")
                nc.vector.tensor_tensor(out=shift[:], in0=scores_col[:], in1=lse[:], op=OP.subtract)
                sneg = wk.tile([10, 1], F32, tag="sneg")
                nc.vector.tensor_scalar(out=sneg[:], in0=scores_col[:], scalar1=NEG,
                                        scalar2=None, op0=OP.add)
                dd = wk.tile([10, 1], F32, tag="dd")
                nc.vector.tensor_tensor(out=dd[:], in0=sneg[:], in1=shift[:], op=OP.subtract)
                fd = wk.tile([10, 1], F32, tag="fd")
                nc.vector.tensor_tensor(out=fd[:], in0=fin_col[:], in1=dd[:], op=OP.mult)
                shf = wk.tile([10, 1], F32, tag="shf")
                nc.vector.tensor_tensor(out=shf[:], in0=shift[:], in1=fd[:], op=OP.add)
                if i == 0:
                    shf2 = wk.tile([10, 1], F32, tag="shf2")
                    nc.vector.tensor_tensor(out=shf2[:], in0=shf[:], in1=negstep0_s[:], op=OP.add)
                    shf = shf2
                shTp = pst.tile([1, 10], F32, tag="ps_ty")
                nc.tensor.transpose(shTp[:], shf[:], i10_s[:])
                shrow = wk.tile([1, 10], F32, tag="shrow")
                nc.vector.tensor_copy(out=shrow[:], in_=shTp[:])
                shbc = pss.tile([128, 10], F32, tag="ps_sm")
                nc.tensor.matmul(shbc[:], onesrow_s[:], shrow[:], start=True, stop=True)
                shbc_s = wk.tile([128, 10], F32, tag="shbcs")
                nc.vector.tensor_copy(out=shbc_s[:], in_=shbc[:])
                cand = wkb.tile([128, 500], F32, tag="cand")
                nc.vector.tensor_tensor(
                    out=cand[:].rearrange("p (t b) -> p t b", t=NTILE),
                    in0=cb[:].rearrange("p (t b) -> p t b", t=NTILE),
                    in1=shbc_s[:].rearrange("p (o b) -> p o b", o=1).to_broadcast([128, NTILE, 10]),
                    op=OP.add)

                # EOS patch on vocab-tile 0 (cols 0:10): only (core0, p==EOS) nonzero
                sb_bc = pss.tile([128, 10], F32, tag="ps_sm")
                nc.tensor.matmul(sb_bc[:], onesrow_s[:], scores_row[:], start=True, stop=True)
                fin_bc = pss.tile([128, 10], F32, tag="ps_sm")
                nc.tensor.matmul(fin_bc[:], onesrow_s[:], fin_row[:], start=True, stop=True)
                w_bc = wk.tile([128, 10], F32, tag="w_bc")
                nc.vector.tensor_scalar(out=w_bc[:], in0=fin_bc[:], scalar1=p2mask[:],
                                        scalar2=None, op0=OP.mult)
                # cand = (1-w)*cand + w*scores  (exact when w in {0,1})
                om = wk.tile([128, 10], F32, tag="om")
                nc.vector.tensor_scalar(out=om[:], in0=w_bc[:], scalar1=-1.0, scalar2=1.0,
                                        op0=OP.mult, op1=OP.add)
                t1p = wk.tile([128, 10], F32, tag="t1p")
                nc.vector.tensor_tensor(out=t1p[:], in0=w_bc[:], in1=sb_bc[:], op=OP.mult)
                t2p = wk.tile([128, 10], F32, tag="t2p")
                nc.vector.tensor_tensor(out=t2p[:], in0=om[:], in1=cand[:, 0:10], op=OP.mult)
                nc.vector.tensor_tensor(out=cand[:, 0:10], in0=t2p[:], in1=t1p[:], op=OP.add)

                vi = wk.tile([128, 16], F32, tag="vi")
                nc.vector.max(vi[:, 0:8], cand[:])
                i8t = wk.tile([128, 8], U32, tag="i8t")
                nc.vector.max_index(i8t[:], vi[:, 0:8], cand[:])
                i8f = wk.tile([128, 8], F32, tag="i8f")
                nc.vector.tensor_copy(out=i8f[:], in_=i8t[:])
                uu = wk.tile([128, 8], F32, tag="uu")
                nc.vector.tensor_scalar(out=uu[:], in0=i8f[:], scalar1=-4.5, scalar2=0.1,
                                        op0=OP.add, op1=OP.mult)
                ti = wk.tile([128, 8], I32, tag="ti")
                nc.vector.tensor_copy(out=ti[:], in_=uu[:])
                tf = wk.tile([128, 8], F32, tag="tf")
                nc.vector.tensor_copy(out=tf[:], in_=ti[:])
                aa = wk.tile([128, 8], F32, tag="aa")
                nc.vector.scalar_tensor_tensor(out=aa[:], in0=i8f[:], scalar=float(V),
                                               in1=offp_s[:].to_broadcast([128, 8]),
                                               op0=OP.mult, op1=OP.add)
                nc.vector.scalar_tensor_tensor(out=vi[:, 8:16], in0=tf[:], scalar=float(128 - 10 * V),
                                               in1=aa[:], op0=OP.mult, op1=OP.add)

                # local collapse -> per-core top-16 (vals+codes)
                Vl = wk.tile([1, 1024], F32, tag="Vl")
                nc.sync.dma_start(out=Vl[:], in_=vi[:, 0:8])
                Cl = wk.tile([1, 1024], F32, tag="Cl")
                nc.sync.dma_start(out=Cl[:], in_=vi[:, 8:16])
                Cl16 = wk.tile([16, 1024], F32, tag="Cl16")
                nc.gpsimd.partition_broadcast(Cl16[:], Cl[:], channels=16)
                lv, lc16 = flat_topk(Vl, Cl16, 1024, "l")
                si = wk.tile([1, 32], F32, tag="si")
                nc.vector.tensor_copy(out=si[:, 0:16], in_=lv[:])
                nc.vector.tensor_copy(out=si[:, 16:32], in_=lc16[0:1, :])
                ib = dr.tile([1, 32], F32, tag="ib")
                nc.sync.dma_start(out=ib[:], in_=si[:])
                ob = dr.tile([NC_, 32], F32, tag="ob")
                nc.gpsimd.collective_compute(
                    "AllGather", OP.bypass,
                    replica_groups=[list(range(NC_))],
                    ins=[ib.opt()], outs=[ob.opt()])
                V1 = wk.tile([1, 128], F32, tag="V1")
                nc.sync.dma_start(out=V1[:], in_=ob[:, 0:16])
                C1 = wk.tile([1, 128], F32, tag="C1")
                nc.sync.dma_start(out=C1[:], in_=ob[:, 16:32])
                C16 = wk.tile([16, 128], F32, tag="C16")
                nc.gpsimd.partition_broadcast(C16[:], C1[:], channels=16)

                h1, c1, hc, c2 = h1n, c1n, hc_n, c2n

            _, _, wvals_fin = merge(nsteps)
            nc.sync.dma_start(out=seqs_o[:], in_=seqs[:])
            nc.sync.dma_start(out=scores_o[:], in_=wvals_fin[:, 0:10])

    nc.finalize()
    return nc


def _prep_inputs(inputs):
    f = lambda x: np.asarray(x, np.float32)
    embedding = f(inputs["embedding"])
    enc_key = f(inputs["enc_key"])
    enc_values = f(inputs["enc_values"])
    W_ih1, W_hh1 = f(inputs["W_ih1"]), f(inputs["W_hh1"])
    b_ih1, b_hh1 = f(inputs["b_ih1"]), f(inputs["b_hh1"])
    W_ih2, W_hh2 = f(inputs["W_ih2"]), f(inputs["W_hh2"])
    b_ih2, b_hh2 = f(inputs["b_ih2"]), f(inputs["b_hh2"])
    Wq, bq = f(inputs["Wq"]), f(inputs["bq"])
    Wc, bc = f(inputs["Wc"]), f(inputs["bc"])

    keyt = np.ascontiguousarray(enc_key[:, 0, :].T)
    valt = np.zeros((128, 16 * 64), np.float32)
    for t in range(16):
        valt[:, 64 * t:64 * t + 64] = enc_values[128 * t:128 * t + 128, 0, :]
    w1a = np.ascontiguousarray(W_ih1[:, 0:128].T)
    w1b = np.concatenate([W_ih1[:, 128:192].T, (b_ih1 + b_hh1)[None, :]], 0)
    w1c = np.ascontiguousarray(W_hh1.T)
    w2a = np.ascontiguousarray(W_ih2.T)
    w2b = np.concatenate([W_hh2.T, (b_ih2 + b_hh2)[None, :]], 0)
    wqt = np.concatenate([Wq.T, bq[None, :]], 0)
    i10 = np.eye(10, dtype=np.float32)
    iota10 = np.tile(np.arange(10, dtype=np.float32)[None, :], (10, 1))
    ones128 = np.ones((128, 1), np.float32)
    onesrow = np.ones((1, 128), np.float32)
    ones1 = np.ones((1, 1), np.float32)
    sosidx = np.full((10, 1), SOS, np.uint32)
    negstep0 = np.zeros((10, 1), np.float32)
    negstep0[1:, 0] = NEG

    WcT_pad = np.zeros((128, NC_ * VS), np.float32)
    WcT_pad[:, :V] = Wc.T
    bc_pad = np.full((NC_ * VS,), -60.0, np.float32)
    bc_pad[:V] = bc

    in_maps = []
    for r in range(NC_):
        sl = slice(r * VS, (r + 1) * VS)
        bshard = bc_pad[sl]
        bct = np.zeros((128, NTILE), np.float32)
        for t in range(NTILE):
            bct[:, t] = bshard[128 * t:128 * t + 128]
        offp = (np.arange(128, dtype=np.float32) + r * VS)[:, None]
        in_maps.append({
            "wct": np.ascontiguousarray(WcT_pad[:, sl]),
            "bct": bct, "offp": offp,
            "keyt": keyt, "valt": valt,
            "w1a": w1a, "w1b": w1b, "w1c": w1c,
            "w2a": w2a, "w2b": w2b, "wqt": wqt,
            "emb": embedding,
            "i10": i10, "iota10": iota10, "ones128": ones128,
            "onesrow": onesrow, "ones1": ones1, "sosidx": sosidx,
            "negstep0": negstep0,
        })
    return in_maps


def _postprocess(seqs_f, scores_row):
    seqs = np.rint(seqs_f).astype(np.int32)
    scores = scores_row[0].astype(np.float32)
    is_eos = seqs[:, 1:] == EOS
    has_eos = is_eos.any(axis=1)
    first = np.argmax(is_eos, axis=1)
    length = np.where(has_eos, first + 2, MAXL + 2).astype(np.float32)
    norm_scores = (scores / (length ** np.float32(ALPHA))).astype(np.float32)
    return seqs, scores, norm_scores


def run(inputs, trace=False):
    nsteps = NSTEPS
    if nsteps not in _cache:
        _cache[nsteps] = _build(nsteps)
    nc = _cache[nsteps]
    in_maps = _prep_inputs(inputs)
    res = bass_utils.run_bass_kernel_spmd(nc, in_maps, list(range(NC_)), trace=trace)
    r0 = res.results[0]
    seqs, scores, norm = _postprocess(r0["seqs_o"], r0["scores_o"])
    return (seqs, scores, norm), res


def kernel(**inputs):
    (seqs, scores, norm), _ = run(inputs, trace=False)
    return seqs, scores, norm
